# revision 20
# baseline (speedup 1.0000x reference)
"""Trainium2 Bass kernel for nn_Attention_35021163332119.

Full multi-head attention: qkv = x @ w_qkv; RoPE(q, k); softmax(q k^T / sqrt(dh)) v;
out = heads @ w_out + b_out.  B=2, N=2048, DIM=1024, H=16, DH=64.

Sharding: 8 cores = (batch b in {0,1}) x (head-group g in {0..3} of 4 heads).
Each core computes its 4 heads end-to-end plus the partial output projection
for its head-group's rows of w_out; the host sums the 4 partials per batch
(bf16 partials, fp32 accumulation) and adds b_out.

Schedule: the kernel is paced by the Scalar engine's softmax exp stream
(~129 us of ACTIVATE at 1 elem/cycle/lane).  The first attention block
(pair 0, i-quarter 0) is fused with the QKV pipeline chunk-by-chunk so the
exp stream starts ~10 us in, and all remaining non-attention PE work (QKV
pair 1, RoPE pair 1, output projection) is drip-fed as small "filler"
pieces into the attention loop so the Tensor engine uses the slack under
the exp stream instead of serializing before/after it.

On-core layout: x is host-transposed to xT [DIM, N]; q,k are produced
transposed ([dh, n], head pairs stacked on 128 partitions); v is produced
in natural [n, dh] layout with an extra ones column so the PV matmul (M=65)
also accumulates the softmax denominator in row 64.  RoPE's interleaved
pair-rotation is a 128x128 +/-1 permutation matmul on the PE plus DVE
multiplies against cos/sin tables.  PSUM->SBUF copies run on Scalar only
while the exp stream has not started; everything later uses DVE.
"""

import numpy as np

B, N, DIM, H, DH = 2, 2048, 1024, 16, 64
ROPE_BASE = 10000.0
SCALE = DH ** -0.5
N_CORES = 8
G = 4                 # heads per core
KT = DIM // 128       # contraction tiles
NT = N // 128         # sequence tiles

_cache = {}


def _rope_tables():
    inv_freq = (1.0 / (ROPE_BASE ** (np.arange(0, DH, 2, dtype=np.float32) / DH)))
    t = np.arange(N, dtype=np.float32)
    freqs = t[:, None] * inv_freq[None, :]          # [N, DH/2]
    freqs = np.repeat(freqs, 2, axis=-1)            # [N, DH] interleaved
    cosT = np.cos(freqs).T.astype(np.float32)       # [DH, N]
    sinT = np.sin(freqs).T.astype(np.float32)
    cos2 = np.concatenate([cosT, cosT], axis=0)     # [128, N] two heads stacked
    sin2 = np.concatenate([sinT, sinT], axis=0)
    return np.ascontiguousarray(cos2), np.ascontiguousarray(sin2)


def _p2t():
    # rot = P2 @ qT with P2 = blockdiag(P, P), P[2t, 2t+1] = -1, P[2t+1, 2t] = 1
    # matmul computes lhsT.T @ rhs, so pass P2.T
    p = np.zeros((DH, DH), dtype=np.float32)
    for t in range(DH // 2):
        p[2 * t, 2 * t + 1] = -1.0
        p[2 * t + 1, 2 * t] = 1.0
    p2 = np.zeros((128, 128), dtype=np.float32)
    p2[:DH, :DH] = p
    p2[DH:, DH:] = p
    return np.ascontiguousarray(p2.T)


def _build():
    if "nc" in _cache:
        return _cache["nc"]

    import concourse.mybir as mybir
    import concourse.tile as tile
    from concourse import bacc

    F32 = mybir.dt.float32
    F32R = mybir.dt.float32r
    BF16 = mybir.dt.bfloat16
    EXP = mybir.ActivationFunctionType.Exp

    nc = bacc.Bacc("TRN2", target_bir_lowering=False, debug=False)
    xT_d = nc.dram_tensor("xT", [DIM, N], BF16, kind="ExternalInput")
    wqk_d = nc.dram_tensor("wqk", [DIM, 4 * 128], BF16, kind="ExternalInput")
    wv_d = nc.dram_tensor("wv", [DIM, G * DH], BF16, kind="ExternalInput")
    wout_d = nc.dram_tensor("wout", [G * DH, DIM], BF16, kind="ExternalInput")
    cos_d = nc.dram_tensor("cos2", [128, N], BF16, kind="ExternalInput")
    sin_d = nc.dram_tensor("sin2", [128, N], BF16, kind="ExternalInput")
    p2t_d = nc.dram_tensor("p2t", [128, 128], BF16, kind="ExternalInput")
    part_d = nc.dram_tensor("part", [N, DIM], BF16, kind="ExternalOutput")

    with tile.TileContext(nc) as tc:
        with tc.tile_pool(name="persist", bufs=1) as persist, \
             tc.tile_pool(name="att", bufs=8) as att, \
             tc.tile_pool(name="norm_w", bufs=2) as norm_w, \
             tc.tile_pool(name="tailw", bufs=1) as tailw, \
             tc.tile_pool(name="outp", bufs=3) as outp, \
             tc.tile_pool(name="xph", bufs=1) as xph, \
             tc.tile_pool(name="rope_w", bufs=2) as rope_w, \
             tc.tile_pool(name="stash", bufs=4) as stash, \
             tc.tile_pool(name="ps", bufs=3, space="PSUM") as ps, \
             tc.tile_pool(name="pso", bufs=2, space="PSUM") as pso:

            # ---- persistent tiles ----
            # bf16 q/k: enables PE fast-weight-load on the scores matmuls
            # (halves the exposed LDWEIGHTS between row-group pairs) and 2x
            # DVE modes on the rope elementwise ops
            qk_sb = [persist.tile([128, N], BF16, tag=f"qk{m}", name=f"qk{m}")
                     for m in range(4)]          # q01T, q23T, k01T, k23T
            v_aug = persist.tile([128, NT, G, DH + 1], BF16, tag="vaug")
            wout_sb = [persist.tile([128, DIM], BF16, tag=f"wo{kk}", name=f"wo{kk}")
                       for kk in range(2)]
            outT = [persist.tile([128, N], BF16, tag=f"outT{p}", name=f"outT{p}")
                    for p in range(2)]

            # ---- phase-1 tiles ----
            xT = [xph.tile([128, N], BF16, tag=f"xT{k}", name=f"xT{k}")
                  for k in range(KT)]
            wqk = [xph.tile([128, 4 * 128], BF16, tag=f"wqk{k}", name=f"wqk{k}")
                   for k in range(KT)]
            wv = [xph.tile([128, G * DH], BF16, tag=f"wv{k}", name=f"wv{k}")
                  for k in range(KT)]
            cos2 = xph.tile([128, N], BF16, tag="cos2")
            sin2 = xph.tile([128, N], BF16, tag="sin2")
            p2t = xph.tile([128, 128], BF16, tag="p2t")
            ones_bc = xph.tile([128, DH], F32, tag="ones_bc")
            warm = xph.tile([128, 8], F32, tag="warm")
            nc.vector.memset(ones_bc, 1.0)

            # preload the exp table set on the Scalar engine during DMA wait
            nc.vector.memset(warm, 0.0)
            nc.scalar.activation(warm, warm, EXP, scale=1.0)

            # ---- input DMAs, ordered so compute can start early ----
            # p2t first: the PE warm-up matmuls below spin on it during the
            # DMA wait so the HAM clock gate opens before real work arrives
            nc.sync.dma_start(out=p2t, in_=p2t_d.ap())
            # interleave wqk[k] with the xT slice it is first used against so
            # the first accumulation chain can start after ~1 MB of traffic
            for k in range(KT):
                nc.sync.dma_start(
                    out=wqk[k],
                    in_=wqk_d.ap().rearrange("(t p) m -> t p m", p=128)[k])
                nc.sync.dma_start(
                    out=xT[k][:, 0:512],
                    in_=xT_d.ap().rearrange(
                        "(t p) n -> t p n", p=128)[k][:, 0:512])
            # cos/sin chunk 0 right away: the first block's rope needs them
            # ~6 us in; in the old order they landed ~22 us and gated the
            # whole exp stream
            nc.sync.dma_start(out=cos2[:, 0:1024], in_=cos_d.ap()[:, 0:1024])
            nc.sync.dma_start(out=sin2[:, 0:1024], in_=sin_d.ap()[:, 0:1024])
            # wv early: PV(0) consumes v tiles 0-1 a couple of us after the
            # first exps
            for k in range(KT):
                nc.sync.dma_start(
                    out=wv[k],
                    in_=wv_d.ap().rearrange("(t p) m -> t p m", p=128)[k])
            for k in range(KT):
                nc.sync.dma_start(
                    out=xT[k][:, 512:1024],
                    in_=xT_d.ap().rearrange(
                        "(t p) n -> t p n", p=128)[k][:, 512:1024])
            # cos/sin chunk 1 before xT chunk 1: rope (m, c2=1) needs both,
            # and the trig tables are 8x smaller
            nc.sync.dma_start(out=cos2[:, 1024:2048], in_=cos_d.ap()[:, 1024:2048])
            nc.sync.dma_start(out=sin2[:, 1024:2048], in_=sin_d.ap()[:, 1024:2048])
            for half in range(2, 4):        # xT chunk 1 (cols 1024:2048)
                hsl = slice(half * 512, (half + 1) * 512)
                for k in range(KT):
                    nc.sync.dma_start(
                        out=xT[k][:, hsl],
                        in_=xT_d.ap().rearrange(
                            "(t p) n -> t p n", p=128)[k][:, hsl])
            for kk in range(2):
                nc.sync.dma_start(
                    out=wout_sb[kk],
                    in_=wout_d.ap().rearrange("(t p) m -> t p m", p=128)[kk])
            nc.vector.memset(v_aug[:, :, :, DH:DH + 1], 1.0)

            # ---- building blocks ----
            def qk_quarter(m, c2, half, kq, holder, on_act):
                """4 accumulating matmuls (k-tiles 4*kq..4*kq+3) for one
                512-wide half of (m, c2); the PSUM tile lives in `holder`
                across the 4 quarters and the SBUF copy happens on the last."""
                if half == 0 and kq == 0:
                    holder.clear()
                    holder.append(ps.tile([128, 1024], F32, tag="s",
                                          name="mm_qk"))
                mm_ps = holder[0]
                csl = slice(c2 * 1024 + half * 512, c2 * 1024 + (half + 1) * 512)
                for k in range(4 * kq, 4 * kq + 4):
                    nc.tensor.matmul(
                        mm_ps[:, half * 512:(half + 1) * 512],
                        wqk[k][:, m * 128:(m + 1) * 128],
                        xT[k][:, csl],
                        start=(k == 0), stop=(k == KT - 1))
                if half == 1 and kq == 1:
                    c2sl = slice(c2 * 1024, (c2 + 1) * 1024)
                    if on_act:
                        nc.scalar.copy(qk_sb[m][:, c2sl], mm_ps)
                    else:
                        nc.vector.tensor_copy(qk_sb[m][:, c2sl], mm_ps)

            def qk_half512(m, c2, half, on_act=True):
                """One self-contained 512-wide half of (m, c2): 8 accumulating
                matmuls into an own PSUM tile + immediate SBUF copy.  Used for
                the startup-critical first block so scores can begin after 16
                matmuls instead of 32."""
                csl = slice(c2 * 1024 + half * 512, c2 * 1024 + (half + 1) * 512)
                mm_ps = ps.tile([128, 1024], F32, tag="s", name="qk512")
                for k in range(KT):
                    nc.tensor.matmul(
                        mm_ps[:, 0:512],
                        wqk[k][:, m * 128:(m + 1) * 128],
                        xT[k][:, csl],
                        start=(k == 0), stop=(k == KT - 1))
                if on_act:
                    nc.scalar.copy(qk_sb[m][:, csl], mm_ps[:, 0:512])
                else:
                    nc.vector.tensor_copy(qk_sb[m][:, csl], mm_ps[:, 0:512])

            def rope_m_half(m, c2, half):
                csl = slice(c2 * 1024 + half * 512, c2 * 1024 + (half + 1) * 512)
                rot_ps = ps.tile([128, 1024], F32, tag="s", name="mm_rot")
                nc.tensor.matmul(rot_ps[:, 0:512], p2t, qk_sb[m][:, csl],
                                 start=True, stop=True)
                tmp = rope_w.tile([128, 1024], BF16, tag="ropetmp")
                nc.vector.tensor_mul(tmp[:, 0:512], rot_ps[:, 0:512],
                                     sin2[:, csl])
                nc.vector.tensor_mul(qk_sb[m][:, csl], qk_sb[m][:, csl],
                                     cos2[:, csl])
                nc.vector.tensor_add(qk_sb[m][:, csl], qk_sb[m][:, csl],
                                     tmp[:, 0:512])

            def rope_m_chunk(m, c2):
                c2sl = slice(c2 * 1024, (c2 + 1) * 1024)
                rot_ps = ps.tile([128, 1024], F32, tag="s", name="mm_rot")
                for half in range(2):
                    csl = slice(c2 * 1024 + half * 512,
                                c2 * 1024 + (half + 1) * 512)
                    nc.tensor.matmul(
                        rot_ps[:, half * 512:(half + 1) * 512],
                        p2t, qk_sb[m][:, csl], start=True, stop=True)
                tmp = rope_w.tile([128, 1024], BF16, tag="ropetmp")
                nc.vector.tensor_mul(tmp, rot_ps, sin2[:, c2sl])
                nc.vector.tensor_mul(qk_sb[m][:, c2sl], qk_sb[m][:, c2sl],
                                     cos2[:, c2sl])
                nc.vector.tensor_add(qk_sb[m][:, c2sl], qk_sb[m][:, c2sl], tmp)

            def v_pair(t2):
                """v for sequence tiles 2*t2, 2*t2+1 in one PSUM tile."""
                mm_ps = ps.tile([128, 1024], F32, tag="s", name="mm_v")
                for sub in range(2):
                    tn = 2 * t2 + sub
                    for k in range(KT):
                        nc.tensor.matmul(
                            mm_ps[:, sub * 256:(sub + 1) * 256],
                            xT[k][:, tn * 128:(tn + 1) * 128],
                            wv[k],
                            start=(k == 0), stop=(k == KT - 1))
                nc.vector.tensor_copy(
                    v_aug[:, 2 * t2:2 * t2 + 2, :, 0:DH],
                    mm_ps[:, 0:512].rearrange("p (t h d) -> p t h d", t=2, h=G))

            # ---- attention machinery ----
            def att_begin():
                return {"o_ps": [pso.tile([DH + 1, 512], F32, tag="o",
                                          name=f"o{hh}") for hh in range(2)],
                        "pends": []}

            def emit_pv(p, st, jj, exps):
                for hh in range(2):
                    for half in range(2):
                        j = 2 * jj + half
                        nc.tensor.matmul(
                            st["o_ps"][hh],
                            v_aug[:, j, 2 * p + hh, :],
                            exps[hh][:, half * 512:(half + 1) * 512],
                            start=(j == 0), stop=(j == NT - 1))

            def att_jj(p, iq, jj, st, fill=None):
                qT, kTt = qk_sb[p], qk_sb[2 + p]
                isl = slice(iq * 512, (iq + 1) * 512)
                s_ps = [ps.tile([128, 1024], F32, tag="s", name=f"s{hh}")
                        for hh in range(2)]
                for half in range(2):
                    j = 2 * jj + half
                    jsl = slice(j * 128, (j + 1) * 128)
                    for hh in range(2):
                        hsl = slice(hh * DH, (hh + 1) * DH)
                        nc.tensor.matmul(
                            s_ps[hh][:, half * 512:(half + 1) * 512],
                            kTt[hsl, jsl], qT[hsl, isl],
                            start=True, stop=True)
                exps = []
                for hh in range(2):
                    expT = att.tile([128, 1024], BF16, tag="exp")
                    nc.scalar.activation(expT, s_ps[hh], EXP, scale=SCALE)
                    exps.append(expT)
                if fill is not None:
                    fill()
                # depth-2 PV pipeline: a PV group is emitted two jj after its
                # exps, so it never waits on the second head's activation
                st["pends"].append((jj, exps))
                if len(st["pends"]) > 2:
                    j0, e0 = st["pends"].pop(0)
                    emit_pv(p, st, j0, e0)

            def att_end(p, iq, st, fast_tail=False):
                for (j0, e0) in st["pends"]:
                    emit_pv(p, st, j0, e0)
                st["pends"] = []
                isl = slice(iq * 512, (iq + 1) * 512)
                # hh=1 first: its chain ends in an SBUF->SBUF DMA hop into
                # outT, which then overlaps hh=0's direct DVE write -- the
                # projection consumers wait on whichever finishes last
                for hh in (1, 0):
                    o_sb = norm_w.tile([DH + 1, 512], F32, tag=f"osb{hh}",
                                       name=f"osb{hh}")
                    nc.vector.tensor_copy(o_sb, st["o_ps"][hh])
                    if fast_tail:
                        # latency-lean: reciprocal in place on partition 64,
                        # broadcast via a K=1 matmul — no DMA hop, no gpsimd
                        nc.vector.reciprocal_approx_fast(
                            o_sb[DH:DH + 1, :], o_sb[DH:DH + 1, :])
                        bc_ps = ps.tile([128, 1024], F32, tag="s", name="bc")
                        nc.tensor.matmul(bc_ps[0:DH, 0:512],
                                         ones_bc[DH:DH + 1, 0:DH],
                                         o_sb[DH:DH + 1, :],
                                         start=True, stop=True)
                        bc = bc_ps[0:DH, 0:512]
                    else:
                        recip0 = norm_w.tile([1, 512], F32, tag=f"r0{hh}",
                                             name=f"r0{hh}")
                        nc.sync.dma_start(out=recip0, in_=o_sb[DH:DH + 1, :])
                        nc.vector.reciprocal_approx_fast(recip0, recip0)
                        bc = norm_w.tile([DH, 512], F32, tag=f"bc{hh}",
                                         name=f"bc{hh}")
                        nc.gpsimd.partition_broadcast(bc, recip0)
                    if hh == 0:
                        nc.vector.tensor_mul(outT[p][0:DH, isl],
                                             o_sb[0:DH, :], bc)
                    else:
                        tmpb = norm_w.tile([DH, 512], BF16, tag="tmpb")
                        nc.vector.tensor_mul(tmpb, o_sb[0:DH, :], bc)
                        nc.sync.dma_start(out=outT[p][DH:2 * DH, isl],
                                          in_=tmpb)

            def att_norm_tail_recips(st):
                """Phase 1 of the final block's latency-lean norm: emit the
                two reciprocals (straight from PSUM) ahead of any other DVE
                work so they're ready when the broadcast matmuls arrive,
                plus bf16 copies of the raw outputs (the final mul may only
                touch one PSUM operand)."""
                recs = []
                for hh in (1, 0):
                    o_sbt = tailw.tile([DH + 1, 512], F32, tag=f"ot{hh}",
                                       name=f"ot{hh}")
                    nc.vector.tensor_copy(o_sbt, st["o_ps"][hh])
                    rec = tailw.tile([1, 512], F32, tag=f"rt{hh}",
                                     name=f"rt{hh}")
                    nc.sync.dma_start(out=rec, in_=o_sbt[DH:DH + 1, :])
                    nc.vector.reciprocal_approx_fast(rec, rec)
                    recs.append((hh, rec, o_sbt))
                return recs

            def att_norm_tail_muls(p, iq, recs):
                """Phase 2: broadcast each reciprocal across partitions via a
                K=1 matmul and write both head halves straight to outT (no
                DMA hop, no gpsimd)."""
                isl = slice(iq * 512, (iq + 1) * 512)
                for hh, rec, o_sbt in recs:
                    bc_ps = ps.tile([128, 1024], F32, tag="s", name="bc")
                    nc.tensor.matmul(bc_ps[0:DH, 0:512],
                                     ones_bc[0:1, 0:DH], rec,
                                     start=True, stop=True)
                    nc.vector.tensor_mul(
                        outT[p][hh * DH:(hh + 1) * DH, isl],
                        o_sbt[0:DH, :], bc_ps[0:DH, 0:512])

            def proj_half(tn, c2, holder):
                """One 512-wide half of projection tile tn; copy+DMA on the
                second half."""
                if c2 == 0:
                    holder.clear()
                    holder.append(ps.tile([128, 1024], F32, tag="s",
                                          name="f_ps"))
                f_ps = holder[0]
                nsl = slice(tn * 128, (tn + 1) * 128)
                c2sl = slice(c2 * 512, (c2 + 1) * 512)
                for kk in range(2):
                    nc.tensor.matmul(
                        f_ps[:, c2sl],
                        outT[kk][:, nsl], wout_sb[kk][:, c2sl],
                        start=(kk == 0), stop=(kk == 1))
                if c2 == 1:
                    out_sb = outp.tile([128, DIM], BF16, tag="osb")
                    nc.vector.tensor_copy(out_sb, f_ps)
                    pdram = part_d.ap().rearrange("(t p) m -> t p m", p=128)[tn]
                    for phh in range(4):
                        psl2 = slice(phh * 32, (phh + 1) * 32)
                        nc.sync.dma_start(out=pdram[psl2, :],
                                          in_=out_sb[psl2, :])

            def proj_tile(tn):
                holder = []
                for c2 in range(2):
                    proj_half(tn, c2, holder)

            # ---- emission ----
            # Scheduling principle: the Tensor engine must stay ~100% dense
            # (its HAM clock gate re-throttles to 1.2 GHz if it idles and
            # never re-warms without a ~3.4us sustained-busy window), while
            # the Scalar engine's exp stream should pace the kernel.  So
            # every attention jj carries a small "filler" slice of the
            # non-attention PE work, slightly oversubscribing the PE.

            # PE warm-up: ~60 tiny matmuls on p2t spin during the input DMA
            # wait so the HAM opens the clock gate before the QKV work lands
            warm_ps = ps.tile([128, 1024], F32, tag="s", name="warm_ps")
            for i in range(60):
                nc.tensor.matmul(warm_ps[:, 0:128], p2t, p2t,
                                 start=(i == 0), stop=(i == 59))

            # First block (pair 0, i-quarter 0): k01/q01 chunk 0 first,
            # k01 chunk 1 mid-block; v rides as per-jj fills since PV(jj)
            # consumes exactly v tiles 2jj, 2jj+1.  q01 chunk 1 is first
            # used by block (0, 2) and is deferred to the filler stream.
            def qk_halfcopy(m, c2, half):
                """8 accumulating matmuls for a 512 half of (m, c2) with the
                SBUF copy right behind them — shortest path to roped q/k for
                the startup-critical first block (copies on Scalar: the exp
                stream has not started yet)."""
                csl = slice(c2 * 1024 + half * 512, c2 * 1024 + (half + 1) * 512)
                mm_ps = ps.tile([128, 1024], F32, tag="s", name="qkh")
                for k in range(KT):
                    nc.tensor.matmul(
                        mm_ps[:, 0:512],
                        wqk[k][:, m * 128:(m + 1) * 128],
                        xT[k][:, csl],
                        start=(k == 0), stop=(k == KT - 1))
                if on_act:
                    nc.scalar.copy(qk_sb[m][:, csl], mm_ps[:, 0:512])
                else:
                    nc.vector.tensor_copy(qk_sb[m][:, csl], mm_ps[:, 0:512])

            st = att_begin()
            hold = []
            for m in (0, 2):
                for half in range(2):
                    for kq in range(2):
                        qk_quarter(m, 0, half, kq, hold, on_act=True)
            for m in (0, 2):
                rope_m_chunk(m, 0)
            att_jj(0, 0, 0, st, fill=(lambda: v_pair(0)))
            att_jj(0, 0, 1, st, fill=(lambda: v_pair(1)))
            for half in range(2):
                for kq in range(2):
                    qk_quarter(2, 1, half, kq, hold, on_act=True)
            rope_m_chunk(2, 1)
            att_jj(0, 0, 2, st, fill=(lambda: v_pair(2)))
            att_jj(0, 0, 3, st, fill=(lambda: v_pair(3)))
            for jj in range(4, 8):
                att_jj(0, 0, jj, st, fill=(lambda t2=jj: v_pair(t2)))
            att_end(0, 0, st)

            # filler pieces, ~2 matmuls each: QKV pair 1 + rope pair 1 and
            # q01 chunk 1; consumed up to 2 per jj across blocks (0,1)-(0,3)
            fillers = []

            def qk_kpair(m, c2, half, kp, holder, last):
                if half == 0 and kp == 0:
                    holder.clear()
                    holder.append(ps.tile([128, 1024], F32, tag="s",
                                          name="qk1"))
                mm_ps = holder[0]
                csl = slice(c2 * 1024 + half * 512,
                            c2 * 1024 + (half + 1) * 512)
                for k in range(2 * kp, 2 * kp + 2):
                    nc.tensor.matmul(
                        mm_ps[:, half * 512:(half + 1) * 512],
                        wqk[k][:, m * 128:(m + 1) * 128],
                        xT[k][:, csl],
                        start=(k == 0), stop=(k == KT - 1))
                if last:
                    c2sl = slice(c2 * 1024, (c2 + 1) * 1024)
                    nc.vector.tensor_copy(qk_sb[m][:, c2sl], mm_ps)

            for (m, c2s) in [(0, [1]), (1, [0, 1]), (3, [0, 1])]:
                for c2 in c2s:
                    h2 = []
                    for half in range(2):
                        for kp in range(4):
                            fillers.append(
                                lambda m=m, c2=c2, half=half, kp=kp, h2=h2:
                                qk_kpair(m, c2, half, kp, h2,
                                         last=(half == 1 and kp == 3)))
                    for half in range(2):
                        fillers.append(
                            lambda m=m, c2=c2, half=half:
                            rope_m_half(m, c2, half))

            for (p, iq) in [(0, 1), (0, 2), (0, 3)]:
                st = att_begin()
                for jj in range(NT // 2):
                    fill = None
                    if jj >= 1 and fillers:
                        pieces = [fillers.pop(0)]
                        if fillers:
                            pieces.append(fillers.pop(0))
                        fill = (lambda ps_=pieces: [f() for f in ps_])
                    att_jj(p, iq, jj, st, fill)
                att_end(p, iq, st)
            # everything pair-1 attention needs must be emitted before its
            # first scores matmul enters the PE queue
            while fillers:
                fillers.pop(0)()

            # two-phase projection for the trailing tiles 12-15: the
            # outT[0]-side accumulation only needs pair-0 results, so it
            # runs as fills inside block (1, 0); after the last block only
            # the outT[1]-side matmuls + add + DMA remain.
            pstash = {}

            def projA(tn):
                nsl = slice(tn * 128, (tn + 1) * 128)
                f_ps = ps.tile([128, 1024], F32, tag="s", name="pA")
                for c2 in range(2):
                    c2sl = slice(c2 * 512, (c2 + 1) * 512)
                    nc.tensor.matmul(f_ps[:, c2sl], outT[0][:, nsl],
                                     wout_sb[0][:, c2sl],
                                     start=True, stop=True)
                sb = stash.tile([128, DIM], F32, tag="pst", name=f"pst{tn}")
                nc.vector.tensor_copy(sb, f_ps)
                pstash[tn] = sb

            def projB(tn):
                nsl = slice(tn * 128, (tn + 1) * 128)
                f_ps = ps.tile([128, 1024], F32, tag="s", name="pB")
                for c2 in range(2):
                    c2sl = slice(c2 * 512, (c2 + 1) * 512)
                    nc.tensor.matmul(f_ps[:, c2sl], outT[1][:, nsl],
                                     wout_sb[1][:, c2sl],
                                     start=True, stop=True)
                # tail-latency lean: add in 512-col halves so the first
                # half's output DMA starts while the second half adds, and
                # split each half across 4 queues (32 KB apiece)
                out_sb = outp.tile([128, DIM], BF16, tag="osb")
                pdram = part_d.ap().rearrange("(t p) m -> t p m", p=128)[tn]
                for c2 in range(2):
                    c2sl = slice(c2 * 512, (c2 + 1) * 512)
                    nc.vector.tensor_add(out_sb[:, c2sl], f_ps[:, c2sl],
                                         pstash[tn][:, c2sl])
                    for phh in range(4):
                        psl2 = slice(phh * 32, (phh + 1) * 32)
                        nc.sync.dma_start(out=pdram[psl2, c2sl],
                                          in_=out_sb[psl2, c2sl])

            st = att_begin()
            for jj in range(NT // 2):
                fill = None
                if 2 <= jj < 6:
                    fill = (lambda tn=10 + jj: projA(tn))
                att_jj(1, 0, jj, st, fill)
            att_end(1, 0, st)

            # projection fillers for tiles 0-11: tile tn needs outT i-chunk
            # tn//4, finished at att_end(1, tn//4); two halves per tile, one
            # per jj starting two jj into the following block
            projq = []
            for (bi, (p, iq)) in enumerate([(1, 1), (1, 2), (1, 3)]):
                for tn in range(4 * bi, 4 * bi + 4):
                    h3 = []
                    for c2 in range(2):
                        projq.append(
                            (bi, lambda tn=tn, c2=c2, h3=h3:
                             proj_half(tn, c2, h3)))
                st = att_begin()
                for jj in range(NT // 2):
                    fill = None
                    if jj >= 2 and projq and projq[0][0] <= bi:
                        fill = projq.pop(0)[1]
                        if jj >= 4 and projq and projq[0][0] < bi:
                            f1, f2 = fill, projq.pop(0)[1]
                            fill = (lambda a=f1, b=f2: (a(), b()))
                    att_jj(p, iq, jj, st, fill)
                if (p, iq) != (1, 3):
                    att_end(p, iq, st)
                else:
                    # final block: drain PV only, then put all remaining
                    # independent PE work BEFORE the norm chain so the
                    # in-order PE queue isn't blocked behind the DVE recips
                    for (j0, e0) in st["pends"]:
                        emit_pv(p, st, j0, e0)
                    st["pends"] = []
            recs = att_norm_tail_recips(st)
            for (_, fn) in projq:
                fn()
            # keep the PE clock warm through the final norm chain so the
            # trailing projection matmuls run at 2.4 GHz
            warm2_ps = ps.tile([128, 1024], F32, tag="s", name="warm2")
            for i in range(12):
                nc.tensor.matmul(warm2_ps[:, 0:128], p2t, p2t,
                                 start=(i == 0), stop=(i == 11))
            att_norm_tail_muls(1, 3, recs)
            for tn in range(12, NT):
                projB(tn)
    nc.compile()
    _cache["nc"] = nc
    return nc


def kernel(x, w_qkv, w_out, b_out, _trace=False):
    import ml_dtypes
    from concourse.bass_utils import run_bass_kernel_spmd

    x = np.asarray(x, dtype=np.float32)
    w_qkv = np.asarray(w_qkv, dtype=np.float32)
    w_out = np.asarray(w_out, dtype=np.float32)
    b_out = np.asarray(b_out, dtype=np.float32)

    cos2, sin2 = _rope_tables()
    p2t = _p2t()

    in_maps = []
    for c in range(N_CORES):
        b, g = divmod(c, G)
        cols = []
        for blk in range(2):                      # q block, k block
            base = blk * H * DH + g * G * DH
            cols.append(w_qkv[:, base:base + G * DH])
        wqk_c = np.ascontiguousarray(np.concatenate(cols, axis=1))  # [DIM, 512]
        wv_c = np.ascontiguousarray(
            w_qkv[:, 2 * H * DH + g * G * DH: 2 * H * DH + (g + 1) * G * DH])
        wout_c = np.ascontiguousarray(
            w_out[g * G * DH:(g + 1) * G * DH, :]).astype(ml_dtypes.bfloat16)
        in_maps.append({
            "xT": np.ascontiguousarray(x[b].T).astype(ml_dtypes.bfloat16),
            "wqk": wqk_c.astype(ml_dtypes.bfloat16),
            "wv": wv_c.astype(ml_dtypes.bfloat16),
            "wout": wout_c,
            "cos2": cos2.astype(ml_dtypes.bfloat16),
            "sin2": sin2.astype(ml_dtypes.bfloat16),
            "p2t": p2t.astype(ml_dtypes.bfloat16),
        })

    nc = _build()
    res = run_bass_kernel_spmd(nc, in_maps, core_ids=list(range(N_CORES)),
                               trace=_trace)
    out = np.empty((B, N, DIM), dtype=np.float32)
    for b in range(B):
        acc = res.results[G * b]["part"].astype(np.float32)
        for g in range(1, G):
            acc += res.results[G * b + g]["part"].astype(np.float32)
        out[b] = acc + b_out
    if _trace:
        kernel.last_results = res
    return out



# revision 25
# speedup vs baseline: 1.0648x; 1.0648x over previous
"""Trainium2 Bass kernel for nn_Attention_35021163332119.

Full multi-head attention: qkv = x @ w_qkv; RoPE(q, k); softmax(q k^T / sqrt(dh)) v;
out = heads @ w_out + b_out.  B=2, N=2048, DIM=1024, H=16, DH=64.

Sharding: 8 cores = (batch b in {0,1}) x (head-group g in {0..3} of 4 heads).
Each core computes its 4 heads end-to-end plus the partial output projection
for its head-group's rows of w_out; the host sums the 4 partials per batch
(bf16 partials, fp32 accumulation) and adds b_out.

Schedule: the kernel is paced by the Scalar engine's softmax exp stream
(~129 us of ACTIVATE at 1 elem/cycle/lane).  The first attention block
(pair 0, i-quarter 0) is fused with the QKV pipeline chunk-by-chunk so the
exp stream starts ~10 us in, and all remaining non-attention PE work (QKV
pair 1, RoPE pair 1, output projection) is drip-fed as small "filler"
pieces into the attention loop so the Tensor engine uses the slack under
the exp stream instead of serializing before/after it.

On-core layout: x is host-transposed to xT [DIM, N]; q,k are produced
transposed ([dh, n], head pairs stacked on 128 partitions); v is produced
in natural [n, dh] layout with an extra ones column so the PV matmul (M=65)
also accumulates the softmax denominator in row 64.  RoPE's interleaved
pair-rotation is a 128x128 +/-1 permutation matmul on the PE plus DVE
multiplies against cos/sin tables.  PSUM->SBUF copies run on Scalar only
while the exp stream has not started; everything later uses DVE.
"""

import numpy as np

B, N, DIM, H, DH = 2, 2048, 1024, 16, 64
ROPE_BASE = 10000.0
SCALE = DH ** -0.5
N_CORES = 8
G = 4                 # heads per core
KT = DIM // 128       # contraction tiles
NT = N // 128         # sequence tiles

_cache = {}


def _rope_tables():
    inv_freq = (1.0 / (ROPE_BASE ** (np.arange(0, DH, 2, dtype=np.float32) / DH)))
    t = np.arange(N, dtype=np.float32)
    freqs = t[:, None] * inv_freq[None, :]          # [N, DH/2]
    freqs = np.repeat(freqs, 2, axis=-1)            # [N, DH] interleaved
    cosT = np.cos(freqs).T.astype(np.float32)       # [DH, N]
    sinT = np.sin(freqs).T.astype(np.float32)
    cos2 = np.concatenate([cosT, cosT], axis=0)     # [128, N] two heads stacked
    sin2 = np.concatenate([sinT, sinT], axis=0)
    return np.ascontiguousarray(cos2), np.ascontiguousarray(sin2)


def _p2t():
    # rot = P2 @ qT with P2 = blockdiag(P, P), P[2t, 2t+1] = -1, P[2t+1, 2t] = 1
    # matmul computes lhsT.T @ rhs, so pass P2.T
    p = np.zeros((DH, DH), dtype=np.float32)
    for t in range(DH // 2):
        p[2 * t, 2 * t + 1] = -1.0
        p[2 * t + 1, 2 * t] = 1.0
    p2 = np.zeros((128, 128), dtype=np.float32)
    p2[:DH, :DH] = p
    p2[DH:, DH:] = p
    return np.ascontiguousarray(p2.T)


def _build():
    if "nc" in _cache:
        return _cache["nc"]

    import concourse.mybir as mybir
    import concourse.tile as tile
    from concourse import bacc

    F32 = mybir.dt.float32
    F32R = mybir.dt.float32r
    BF16 = mybir.dt.bfloat16
    EXP = mybir.ActivationFunctionType.Exp

    nc = bacc.Bacc("TRN2", target_bir_lowering=False, debug=False)
    xT_d = nc.dram_tensor("xT", [DIM, N], BF16, kind="ExternalInput")
    wqk_d = nc.dram_tensor("wqk", [DIM, 4 * 128], BF16, kind="ExternalInput")
    wv_d = nc.dram_tensor("wv", [DIM, G * DH], BF16, kind="ExternalInput")
    wout_d = nc.dram_tensor("wout", [G * DH, DIM], BF16, kind="ExternalInput")
    cos_d = nc.dram_tensor("cos2", [128, N], BF16, kind="ExternalInput")
    sin_d = nc.dram_tensor("sin2", [128, N], BF16, kind="ExternalInput")
    p2t_d = nc.dram_tensor("p2t", [128, 128], BF16, kind="ExternalInput")
    part_d = nc.dram_tensor("part", [N, DIM], BF16, kind="ExternalOutput")

    with tile.TileContext(nc) as tc:
        with tc.tile_pool(name="persist", bufs=1) as persist, \
             tc.tile_pool(name="att", bufs=8) as att, \
             tc.tile_pool(name="norm_w", bufs=2) as norm_w, \
             tc.tile_pool(name="tailw", bufs=1) as tailw, \
             tc.tile_pool(name="outp", bufs=3) as outp, \
             tc.tile_pool(name="xph", bufs=1) as xph, \
             tc.tile_pool(name="rope_w", bufs=2) as rope_w, \
             tc.tile_pool(name="stash", bufs=4) as stash, \
             tc.tile_pool(name="ps", bufs=3, space="PSUM") as ps, \
             tc.tile_pool(name="pso", bufs=2, space="PSUM") as pso:

            # ---- persistent tiles ----
            # bf16 q/k: enables PE fast-weight-load on the scores matmuls
            # (halves the exposed LDWEIGHTS between row-group pairs) and 2x
            # DVE modes on the rope elementwise ops
            qk_sb = [persist.tile([128, N], BF16, tag=f"qk{m}", name=f"qk{m}")
                     for m in range(4)]          # q01T, q23T, k01T, k23T
            v_aug = persist.tile([128, NT, G, DH + 1], BF16, tag="vaug")
            wout_sb = [persist.tile([128, DIM], BF16, tag=f"wo{kk}", name=f"wo{kk}")
                       for kk in range(2)]
            outT = [persist.tile([128, N], BF16, tag=f"outT{p}", name=f"outT{p}")
                    for p in range(2)]

            # ---- phase-1 tiles ----
            xT = [xph.tile([128, N], BF16, tag=f"xT{k}", name=f"xT{k}")
                  for k in range(KT)]
            wqk = [xph.tile([128, 4 * 128], BF16, tag=f"wqk{k}", name=f"wqk{k}")
                   for k in range(KT)]
            wv = [xph.tile([128, G * DH], BF16, tag=f"wv{k}", name=f"wv{k}")
                  for k in range(KT)]
            cos2 = xph.tile([128, N], BF16, tag="cos2")
            sin2 = xph.tile([128, N], BF16, tag="sin2")
            p2t = xph.tile([128, 128], BF16, tag="p2t")
            ones_bc = xph.tile([128, DH], F32, tag="ones_bc")
            warm = xph.tile([128, 8], F32, tag="warm")
            nc.vector.memset(ones_bc, 1.0)

            # preload the exp table set on the Scalar engine during DMA wait
            nc.vector.memset(warm, 0.0)
            nc.scalar.activation(warm, warm, EXP, scale=1.0)

            # ---- input DMAs, ordered so compute can start early ----
            # The Sync sequencer spends ~0.6 us ISSUING each dma_start and is
            # saturated during the input phase, so the count on Sync is kept
            # minimal and the late bulk (xT half 1 / chunk 1, trailing trig,
            # wout) is issued from the otherwise-idle GpSimd sequencer in
            # parallel.
            # p2t first: the PE warm-up matmuls below spin on it during the
            # DMA wait so the HAM clock gate opens before real work arrives
            nc.sync.dma_start(out=p2t, in_=p2t_d.ap())
            # interleave wqk[k] with the xT slice it is first used against so
            # the first accumulation chain can start after ~1 MB of traffic
            for k in range(KT):
                nc.sync.dma_start(
                    out=wqk[k],
                    in_=wqk_d.ap().rearrange("(t p) m -> t p m", p=128)[k])
                nc.sync.dma_start(
                    out=xT[k][:, 0:512],
                    in_=xT_d.ap().rearrange(
                        "(t p) n -> t p n", p=128)[k][:, 0:512])
            # trig for the first 512 columns right behind the chunk the first
            # block's rope consumes
            nc.sync.dma_start(out=cos2[:, 0:512], in_=cos_d.ap()[:, 0:512])
            nc.sync.dma_start(out=sin2[:, 0:512], in_=sin_d.ap()[:, 0:512])
            for k in range(KT):
                nc.sync.dma_start(
                    out=xT[k][:, 512:1024],
                    in_=xT_d.ap().rearrange(
                        "(t p) n -> t p n", p=128)[k][:, 512:1024])
            nc.sync.dma_start(out=cos2[:, 512:2048], in_=cos_d.ap()[:, 512:2048])
            nc.sync.dma_start(out=sin2[:, 512:2048], in_=sin_d.ap()[:, 512:2048])
            for k in range(KT):
                nc.sync.dma_start(
                    out=wv[k],
                    in_=wv_d.ap().rearrange("(t p) m -> t p m", p=128)[k])
            # xT chunk 1 + wout issue from the (idle) GpSimd sequencer so the
            # Sync queue drains faster; a tiny copy dependent on the last
            # half-0 slice holds them back so their transfers don't steal
            # HBM bandwidth from the startup-critical half-0 stream
            nc.gpsimd.tensor_copy(warm[:, 0:1], xT[KT - 1][:, 511:512])
            for half in range(2, 4):        # xT chunk 1 (cols 1024:2048)
                hsl = slice(half * 512, (half + 1) * 512)
                for k in range(KT):
                    nc.gpsimd.dma_start(
                        out=xT[k][:, hsl],
                        in_=xT_d.ap().rearrange(
                            "(t p) n -> t p n", p=128)[k][:, hsl])
            for kk in range(2):
                nc.gpsimd.dma_start(
                    out=wout_sb[kk],
                    in_=wout_d.ap().rearrange("(t p) m -> t p m", p=128)[kk])
            nc.vector.memset(v_aug[:, :, :, DH:DH + 1], 1.0)

            # ---- building blocks ----
            def qk_quarter(m, c2, half, kq, holder, on_act):
                """4 accumulating matmuls (k-tiles 4*kq..4*kq+3) for one
                512-wide half of (m, c2); the PSUM tile lives in `holder`
                across the 4 quarters and the SBUF copy happens on the last."""
                if half == 0 and kq == 0:
                    holder.clear()
                    holder.append(ps.tile([128, 1024], F32, tag="s",
                                          name="mm_qk"))
                mm_ps = holder[0]
                csl = slice(c2 * 1024 + half * 512, c2 * 1024 + (half + 1) * 512)
                for k in range(4 * kq, 4 * kq + 4):
                    nc.tensor.matmul(
                        mm_ps[:, half * 512:(half + 1) * 512],
                        wqk[k][:, m * 128:(m + 1) * 128],
                        xT[k][:, csl],
                        start=(k == 0), stop=(k == KT - 1))
                if half == 1 and kq == 1:
                    c2sl = slice(c2 * 1024, (c2 + 1) * 1024)
                    if on_act:
                        nc.scalar.copy(qk_sb[m][:, c2sl], mm_ps)
                    else:
                        nc.vector.tensor_copy(qk_sb[m][:, c2sl], mm_ps)

            def qk_half512(m, c2, half, on_act=True):
                """One self-contained 512-wide half of (m, c2): 8 accumulating
                matmuls into an own PSUM tile + immediate SBUF copy.  Used for
                the startup-critical first block so scores can begin after 16
                matmuls instead of 32."""
                csl = slice(c2 * 1024 + half * 512, c2 * 1024 + (half + 1) * 512)
                mm_ps = ps.tile([128, 1024], F32, tag="s", name="qk512")
                for k in range(KT):
                    nc.tensor.matmul(
                        mm_ps[:, 0:512],
                        wqk[k][:, m * 128:(m + 1) * 128],
                        xT[k][:, csl],
                        start=(k == 0), stop=(k == KT - 1))
                if on_act:
                    nc.scalar.copy(qk_sb[m][:, csl], mm_ps[:, 0:512])
                else:
                    nc.vector.tensor_copy(qk_sb[m][:, csl], mm_ps[:, 0:512])

            def rope_m_half(m, c2, half):
                csl = slice(c2 * 1024 + half * 512, c2 * 1024 + (half + 1) * 512)
                rot_ps = ps.tile([128, 1024], F32, tag="s", name="mm_rot")
                nc.tensor.matmul(rot_ps[:, 0:512], p2t, qk_sb[m][:, csl],
                                 start=True, stop=True)
                tmp = rope_w.tile([128, 1024], BF16, tag="ropetmp")
                nc.vector.tensor_mul(tmp[:, 0:512], rot_ps[:, 0:512],
                                     sin2[:, csl])
                nc.vector.tensor_mul(qk_sb[m][:, csl], qk_sb[m][:, csl],
                                     cos2[:, csl])
                nc.vector.tensor_add(qk_sb[m][:, csl], qk_sb[m][:, csl],
                                     tmp[:, 0:512])

            def rope_m_chunk(m, c2):
                c2sl = slice(c2 * 1024, (c2 + 1) * 1024)
                rot_ps = ps.tile([128, 1024], F32, tag="s", name="mm_rot")
                for half in range(2):
                    csl = slice(c2 * 1024 + half * 512,
                                c2 * 1024 + (half + 1) * 512)
                    nc.tensor.matmul(
                        rot_ps[:, half * 512:(half + 1) * 512],
                        p2t, qk_sb[m][:, csl], start=True, stop=True)
                tmp = rope_w.tile([128, 1024], BF16, tag="ropetmp")
                nc.vector.tensor_mul(tmp, rot_ps, sin2[:, c2sl])
                nc.vector.tensor_mul(qk_sb[m][:, c2sl], qk_sb[m][:, c2sl],
                                     cos2[:, c2sl])
                nc.vector.tensor_add(qk_sb[m][:, c2sl], qk_sb[m][:, c2sl], tmp)

            def v_pair(t2):
                """v for sequence tiles 2*t2, 2*t2+1 in one PSUM tile."""
                mm_ps = ps.tile([128, 1024], F32, tag="s", name="mm_v")
                for sub in range(2):
                    tn = 2 * t2 + sub
                    for k in range(KT):
                        nc.tensor.matmul(
                            mm_ps[:, sub * 256:(sub + 1) * 256],
                            xT[k][:, tn * 128:(tn + 1) * 128],
                            wv[k],
                            start=(k == 0), stop=(k == KT - 1))
                nc.vector.tensor_copy(
                    v_aug[:, 2 * t2:2 * t2 + 2, :, 0:DH],
                    mm_ps[:, 0:512].rearrange("p (t h d) -> p t h d", t=2, h=G))

            # ---- attention machinery ----
            def att_begin():
                return {"o_ps": [pso.tile([DH + 1, 512], F32, tag="o",
                                          name=f"o{hh}") for hh in range(2)],
                        "pends": []}

            def emit_pv(p, st, jj, exps):
                for hh in range(2):
                    for half in range(2):
                        j = 2 * jj + half
                        nc.tensor.matmul(
                            st["o_ps"][hh],
                            v_aug[:, j, 2 * p + hh, :],
                            exps[hh][:, half * 512:(half + 1) * 512],
                            start=(j == 0), stop=(j == NT - 1))

            def att_jj(p, iq, jj, st, fill=None):
                qT, kTt = qk_sb[p], qk_sb[2 + p]
                isl = slice(iq * 512, (iq + 1) * 512)
                s_ps = [ps.tile([128, 1024], F32, tag="s", name=f"s{hh}")
                        for hh in range(2)]
                for half in range(2):
                    j = 2 * jj + half
                    jsl = slice(j * 128, (j + 1) * 128)
                    for hh in range(2):
                        hsl = slice(hh * DH, (hh + 1) * DH)
                        nc.tensor.matmul(
                            s_ps[hh][:, half * 512:(half + 1) * 512],
                            kTt[hsl, jsl], qT[hsl, isl],
                            start=True, stop=True)
                exps = []
                for hh in range(2):
                    expT = att.tile([128, 1024], BF16, tag="exp")
                    nc.scalar.activation(expT, s_ps[hh], EXP, scale=SCALE)
                    exps.append(expT)
                if fill is not None:
                    fill()
                # depth-2 PV pipeline: a PV group is emitted two jj after its
                # exps, so it never waits on the second head's activation
                st["pends"].append((jj, exps))
                if len(st["pends"]) > 2:
                    j0, e0 = st["pends"].pop(0)
                    emit_pv(p, st, j0, e0)

            def att_end(p, iq, st, fast_tail=False):
                for (j0, e0) in st["pends"]:
                    emit_pv(p, st, j0, e0)
                st["pends"] = []
                isl = slice(iq * 512, (iq + 1) * 512)
                # hh=1 first: its chain ends in an SBUF->SBUF DMA hop into
                # outT, which then overlaps hh=0's direct DVE write -- the
                # projection consumers wait on whichever finishes last
                for hh in (1, 0):
                    o_sb = norm_w.tile([DH + 1, 512], F32, tag=f"osb{hh}",
                                       name=f"osb{hh}")
                    nc.vector.tensor_copy(o_sb, st["o_ps"][hh])
                    if fast_tail:
                        # latency-lean: reciprocal in place on partition 64,
                        # broadcast via a K=1 matmul — no DMA hop, no gpsimd
                        nc.vector.reciprocal_approx_fast(
                            o_sb[DH:DH + 1, :], o_sb[DH:DH + 1, :])
                        bc_ps = ps.tile([128, 1024], F32, tag="s", name="bc")
                        nc.tensor.matmul(bc_ps[0:DH, 0:512],
                                         ones_bc[DH:DH + 1, 0:DH],
                                         o_sb[DH:DH + 1, :],
                                         start=True, stop=True)
                        bc = bc_ps[0:DH, 0:512]
                    else:
                        recip0 = norm_w.tile([1, 512], F32, tag=f"r0{hh}",
                                             name=f"r0{hh}")
                        nc.sync.dma_start(out=recip0, in_=o_sb[DH:DH + 1, :])
                        nc.vector.reciprocal_approx_fast(recip0, recip0)
                        bc = norm_w.tile([DH, 512], F32, tag=f"bc{hh}",
                                         name=f"bc{hh}")
                        nc.gpsimd.partition_broadcast(bc, recip0)
                    if hh == 0:
                        nc.vector.tensor_mul(outT[p][0:DH, isl],
                                             o_sb[0:DH, :], bc)
                    else:
                        tmpb = norm_w.tile([DH, 512], BF16, tag="tmpb")
                        nc.vector.tensor_mul(tmpb, o_sb[0:DH, :], bc)
                        nc.sync.dma_start(out=outT[p][DH:2 * DH, isl],
                                          in_=tmpb)

            def att_norm_tail_recips(st):
                """Phase 1 of the final block's latency-lean norm: emit the
                two reciprocals (straight from PSUM) ahead of any other DVE
                work so they're ready when the broadcast matmuls arrive,
                plus bf16 copies of the raw outputs (the final mul may only
                touch one PSUM operand)."""
                recs = []
                for hh in (1, 0):
                    o_sbt = tailw.tile([DH + 1, 512], F32, tag=f"ot{hh}",
                                       name=f"ot{hh}")
                    nc.vector.tensor_copy(o_sbt, st["o_ps"][hh])
                    rec = tailw.tile([1, 512], F32, tag=f"rt{hh}",
                                     name=f"rt{hh}")
                    nc.sync.dma_start(out=rec, in_=o_sbt[DH:DH + 1, :])
                    nc.vector.reciprocal_approx_fast(rec, rec)
                    recs.append((hh, rec, o_sbt))
                return recs

            def att_norm_tail_muls(p, iq, recs):
                """Phase 2: broadcast each reciprocal across partitions via a
                K=1 matmul and write both head halves straight to outT (no
                DMA hop, no gpsimd)."""
                isl = slice(iq * 512, (iq + 1) * 512)
                for hh, rec, o_sbt in recs:
                    bc_ps = ps.tile([128, 1024], F32, tag="s", name="bc")
                    nc.tensor.matmul(bc_ps[0:DH, 0:512],
                                     ones_bc[0:1, 0:DH], rec,
                                     start=True, stop=True)
                    nc.vector.tensor_mul(
                        outT[p][hh * DH:(hh + 1) * DH, isl],
                        o_sbt[0:DH, :], bc_ps[0:DH, 0:512])

            def proj_half(tn, c2, holder):
                """One 512-wide half of projection tile tn; copy+DMA on the
                second half."""
                if c2 == 0:
                    holder.clear()
                    holder.append(ps.tile([128, 1024], F32, tag="s",
                                          name="f_ps"))
                f_ps = holder[0]
                nsl = slice(tn * 128, (tn + 1) * 128)
                c2sl = slice(c2 * 512, (c2 + 1) * 512)
                for kk in range(2):
                    nc.tensor.matmul(
                        f_ps[:, c2sl],
                        outT[kk][:, nsl], wout_sb[kk][:, c2sl],
                        start=(kk == 0), stop=(kk == 1))
                if c2 == 1:
                    out_sb = outp.tile([128, DIM], BF16, tag="osb")
                    nc.vector.tensor_copy(out_sb, f_ps)
                    pdram = part_d.ap().rearrange("(t p) m -> t p m", p=128)[tn]
                    for phh in range(2):
                        psl2 = slice(phh * 64, (phh + 1) * 64)
                        nc.sync.dma_start(out=pdram[psl2, :],
                                          in_=out_sb[psl2, :])

            def proj_tile(tn):
                holder = []
                for c2 in range(2):
                    proj_half(tn, c2, holder)

            # ---- emission ----
            # Scheduling principle: the Tensor engine must stay ~100% dense
            # (its HAM clock gate re-throttles to 1.2 GHz if it idles and
            # never re-warms without a ~3.4us sustained-busy window), while
            # the Scalar engine's exp stream should pace the kernel.  So
            # every attention jj carries a small "filler" slice of the
            # non-attention PE work, slightly oversubscribing the PE.

            # PE warm-up: ~60 tiny matmuls on p2t spin during the input DMA
            # wait so the HAM opens the clock gate before the QKV work lands
            warm_ps = ps.tile([128, 1024], F32, tag="s", name="warm_ps")
            for i in range(60):
                nc.tensor.matmul(warm_ps[:, 0:128], p2t, p2t,
                                 start=(i == 0), stop=(i == 59))

            # First block (pair 0, i-quarter 0): k01/q01 chunk 0 first,
            # k01 chunk 1 mid-block; v rides as per-jj fills since PV(jj)
            # consumes exactly v tiles 2jj, 2jj+1.  q01 chunk 1 is first
            # used by block (0, 2) and is deferred to the filler stream.
            def qk_halfcopy(m, c2, half):
                """8 accumulating matmuls for a 512 half of (m, c2) with the
                SBUF copy right behind them — shortest path to roped q/k for
                the startup-critical first block (copies on Scalar: the exp
                stream has not started yet)."""
                csl = slice(c2 * 1024 + half * 512, c2 * 1024 + (half + 1) * 512)
                mm_ps = ps.tile([128, 1024], F32, tag="s", name="qkh")
                for k in range(KT):
                    nc.tensor.matmul(
                        mm_ps[:, 0:512],
                        wqk[k][:, m * 128:(m + 1) * 128],
                        xT[k][:, csl],
                        start=(k == 0), stop=(k == KT - 1))
                if on_act:
                    nc.scalar.copy(qk_sb[m][:, csl], mm_ps[:, 0:512])
                else:
                    nc.vector.tensor_copy(qk_sb[m][:, csl], mm_ps[:, 0:512])

            st = att_begin()
            hold = []
            for m in (0, 2):
                for half in range(2):
                    for kq in range(2):
                        qk_quarter(m, 0, half, kq, hold, on_act=True)
            for m in (0, 2):
                rope_m_chunk(m, 0)
            att_jj(0, 0, 0, st, fill=(lambda: v_pair(0)))
            att_jj(0, 0, 1, st, fill=(lambda: v_pair(1)))
            for half in range(2):
                for kq in range(2):
                    qk_quarter(2, 1, half, kq, hold, on_act=True)
            rope_m_chunk(2, 1)
            att_jj(0, 0, 2, st, fill=(lambda: v_pair(2)))
            att_jj(0, 0, 3, st, fill=(lambda: v_pair(3)))
            for jj in range(4, 8):
                att_jj(0, 0, jj, st, fill=(lambda t2=jj: v_pair(t2)))
            att_end(0, 0, st)

            # filler pieces, ~2 matmuls each: QKV pair 1 + rope pair 1 and
            # q01 chunk 1; consumed up to 2 per jj across blocks (0,1)-(0,3)
            fillers = []

            def qk_kpair(m, c2, half, kp, holder, last):
                if half == 0 and kp == 0:
                    holder.clear()
                    holder.append(ps.tile([128, 1024], F32, tag="s",
                                          name="qk1"))
                mm_ps = holder[0]
                csl = slice(c2 * 1024 + half * 512,
                            c2 * 1024 + (half + 1) * 512)
                for k in range(2 * kp, 2 * kp + 2):
                    nc.tensor.matmul(
                        mm_ps[:, half * 512:(half + 1) * 512],
                        wqk[k][:, m * 128:(m + 1) * 128],
                        xT[k][:, csl],
                        start=(k == 0), stop=(k == KT - 1))
                if last:
                    c2sl = slice(c2 * 1024, (c2 + 1) * 1024)
                    nc.vector.tensor_copy(qk_sb[m][:, c2sl], mm_ps)

            for (m, c2s) in [(0, [1]), (1, [0, 1]), (3, [0, 1])]:
                for c2 in c2s:
                    h2 = []
                    for half in range(2):
                        for kp in range(4):
                            fillers.append(
                                lambda m=m, c2=c2, half=half, kp=kp, h2=h2:
                                qk_kpair(m, c2, half, kp, h2,
                                         last=(half == 1 and kp == 3)))
                    for half in range(2):
                        fillers.append(
                            lambda m=m, c2=c2, half=half:
                            rope_m_half(m, c2, half))

            for (p, iq) in [(0, 1), (0, 2), (0, 3)]:
                st = att_begin()
                for jj in range(NT // 2):
                    fill = None
                    if jj >= 1 and fillers:
                        pieces = [fillers.pop(0)]
                        if fillers:
                            pieces.append(fillers.pop(0))
                        fill = (lambda ps_=pieces: [f() for f in ps_])
                    att_jj(p, iq, jj, st, fill)
                att_end(p, iq, st)
            # everything pair-1 attention needs must be emitted before its
            # first scores matmul enters the PE queue
            while fillers:
                fillers.pop(0)()

            # two-phase projection for the trailing tiles 12-15: the
            # outT[0]-side accumulation only needs pair-0 results, so it
            # runs as fills inside block (1, 0); after the last block only
            # the outT[1]-side matmuls + add + DMA remain.
            pstash = {}

            def projA(tn):
                nsl = slice(tn * 128, (tn + 1) * 128)
                f_ps = ps.tile([128, 1024], F32, tag="s", name="pA")
                for c2 in range(2):
                    c2sl = slice(c2 * 512, (c2 + 1) * 512)
                    nc.tensor.matmul(f_ps[:, c2sl], outT[0][:, nsl],
                                     wout_sb[0][:, c2sl],
                                     start=True, stop=True)
                sb = stash.tile([128, DIM], F32, tag="pst", name=f"pst{tn}")
                nc.vector.tensor_copy(sb, f_ps)
                pstash[tn] = sb

            def projB(tn):
                nsl = slice(tn * 128, (tn + 1) * 128)
                f_ps = ps.tile([128, 1024], F32, tag="s", name="pB")
                for c2 in range(2):
                    c2sl = slice(c2 * 512, (c2 + 1) * 512)
                    nc.tensor.matmul(f_ps[:, c2sl], outT[1][:, nsl],
                                     wout_sb[1][:, c2sl],
                                     start=True, stop=True)
                # tail-latency lean: 4 row-contiguous 64 KB output chunks,
                # each issued from a different engine sequencer (a dma_start
                # costs ~0.6 us of issue time and Sync alone can't keep up
                # at the tail; Scalar/Tensor/GpSimd are idle by now)
                out_sb = outp.tile([128, DIM], BF16, tag="osb")
                nc.vector.tensor_add(out_sb, f_ps, pstash[tn])
                pdram = part_d.ap().rearrange("(t p) m -> t p m", p=128)[tn]
                for phh, eng in enumerate((nc.scalar, nc.gpsimd,
                                           nc.sync, nc.scalar)):
                    psl2 = slice(phh * 32, (phh + 1) * 32)
                    eng.dma_start(out=pdram[psl2, :], in_=out_sb[psl2, :])

            st = att_begin()
            for jj in range(NT // 2):
                fill = None
                if 2 <= jj < 6:
                    fill = (lambda tn=10 + jj: projA(tn))
                att_jj(1, 0, jj, st, fill)
            att_end(1, 0, st)

            # projection fillers for tiles 0-11: tile tn needs outT i-chunk
            # tn//4, finished at att_end(1, tn//4); two halves per tile, one
            # per jj starting two jj into the following block
            projq = []
            for (bi, (p, iq)) in enumerate([(1, 1), (1, 2), (1, 3)]):
                for tn in range(4 * bi, 4 * bi + 4):
                    h3 = []
                    for c2 in range(2):
                        projq.append(
                            (bi, lambda tn=tn, c2=c2, h3=h3:
                             proj_half(tn, c2, h3)))
                st = att_begin()
                for jj in range(NT // 2):
                    fill = None
                    if jj >= 2 and projq and projq[0][0] <= bi:
                        fill = projq.pop(0)[1]
                        if jj >= 4 and projq and projq[0][0] < bi:
                            f1, f2 = fill, projq.pop(0)[1]
                            fill = (lambda a=f1, b=f2: (a(), b()))
                    att_jj(p, iq, jj, st, fill)
                if (p, iq) != (1, 3):
                    att_end(p, iq, st)
                else:
                    # final block: drain PV only, then put all remaining
                    # independent PE work BEFORE the norm chain so the
                    # in-order PE queue isn't blocked behind the DVE recips
                    for (j0, e0) in st["pends"]:
                        emit_pv(p, st, j0, e0)
                    st["pends"] = []
            recs = att_norm_tail_recips(st)
            for (_, fn) in projq:
                fn()
            # keep the PE clock warm through the final norm chain so the
            # trailing projection matmuls run at 2.4 GHz
            warm2_ps = ps.tile([128, 1024], F32, tag="s", name="warm2")
            for i in range(12):
                nc.tensor.matmul(warm2_ps[:, 0:128], p2t, p2t,
                                 start=(i == 0), stop=(i == 11))
            att_norm_tail_muls(1, 3, recs)
            for tn in range(12, NT):
                projB(tn)
    nc.compile()
    _cache["nc"] = nc
    return nc


def kernel(x, w_qkv, w_out, b_out, _trace=False):
    import ml_dtypes
    from concourse.bass_utils import run_bass_kernel_spmd

    x = np.asarray(x, dtype=np.float32)
    w_qkv = np.asarray(w_qkv, dtype=np.float32)
    w_out = np.asarray(w_out, dtype=np.float32)
    b_out = np.asarray(b_out, dtype=np.float32)

    cos2, sin2 = _rope_tables()
    p2t = _p2t()

    in_maps = []
    for c in range(N_CORES):
        b, g = divmod(c, G)
        cols = []
        for blk in range(2):                      # q block, k block
            base = blk * H * DH + g * G * DH
            cols.append(w_qkv[:, base:base + G * DH])
        wqk_c = np.ascontiguousarray(np.concatenate(cols, axis=1))  # [DIM, 512]
        wv_c = np.ascontiguousarray(
            w_qkv[:, 2 * H * DH + g * G * DH: 2 * H * DH + (g + 1) * G * DH])
        wout_c = np.ascontiguousarray(
            w_out[g * G * DH:(g + 1) * G * DH, :]).astype(ml_dtypes.bfloat16)
        in_maps.append({
            "xT": np.ascontiguousarray(x[b].T).astype(ml_dtypes.bfloat16),
            "wqk": wqk_c.astype(ml_dtypes.bfloat16),
            "wv": wv_c.astype(ml_dtypes.bfloat16),
            "wout": wout_c,
            "cos2": cos2.astype(ml_dtypes.bfloat16),
            "sin2": sin2.astype(ml_dtypes.bfloat16),
            "p2t": p2t.astype(ml_dtypes.bfloat16),
        })

    nc = _build()
    res = run_bass_kernel_spmd(nc, in_maps, core_ids=list(range(N_CORES)),
                               trace=_trace)
    out = np.empty((B, N, DIM), dtype=np.float32)
    for b in range(B):
        acc = res.results[G * b]["part"].astype(np.float32)
        for g in range(1, G):
            acc += res.results[G * b + g]["part"].astype(np.float32)
        out[b] = acc + b_out
    if _trace:
        kernel.last_results = res
    return out

